# revision 21
# baseline (speedup 1.0000x reference)
"""Trainium2 Bass kernel for DyanEncoder (FISTA sparse coding).

reference computes:
  dic = make_dictionary(rr, theta, 36)            # [36, 161]
  C   = fista(dic, x, lam=0.1, 100 iters)         # [64, 161, 384]
  reconst = einsum('tp,npd->ntd', dic, C)         # [64, 36, 384]
  returns (C, dic, reconst)

The reference's convergence early-exit (TOL=1e-5 on the global update norm)
never fires for randn-scale inputs: the update norm plateaus around 1e-2,
1000x above TOL. So the computation is exactly 100 unfrozen FISTA steps and
needs no cross-core collective; we shard data-parallel over the batch
(64 samples -> 8 samples/core, 8 cores).

Per-core formulation (y-form FISTA, columns = 8 samples x 384 dims = 3072):
  z_k   = A @ y_{k-1} + B          A = I - Linv*DtD [161,161], in PSUM
  c_k   = clip(z_k, +-thr)         (softshrink x = z - clip(z))
  x_k   = z_k - c_k
  y_k   = (1+tt_k) x_k - tt_k x_{k-1}
B = Linv * D^T Ys is folded into the matmul by stacking Linv*D rows under A
and the constant Ys rows under y (contract dim 161+36=197 -> K-tiles
{128, 69}); out dim 161 -> M-tiles {128, 33}.

Engine split per chunk (512 cols, 6 chunks):
  PE:     4 fp32 matmuls (2 K-tiles x 2 M-tiles), accumulate in PSUM
  DVE:    c = tensor_scalar(z, min thr, max -thr);  x = z - c (STT from PSUM)
  ACT:    s = (tt_{k+1}/(1+tt_{k+1})) * x  (scaled copy) + tiny per-iteration
          weight copies scaled by (1+tt_k) -- GPSIMD's Pool engine has no
          scalar_tensor_tensor opcode, so the momentum scalars are folded
          into the stationary weights and the pre-scaled s
  GPSIMD: y~ = x - s  (plain tensor_sub; A-weights carry the (1+tt) factor)
fp32 matmul (4 cycles/row) is required: float32r is a 1-8-11 format whose
2.4e-4 input rounding is amplified ~200x by the 100 unconverged iterations
(tf32-proxy measured 25% final error; bf16 79%). The kernel is PE-streaming
bound at ~98% of the fp32 matmul roofline (TimelineSim: 2.083 ms vs 2.048 ms
floor = 2400 matmuls x 512 rows x 4 cyc @ 2.4 GHz).
"""

import numpy as np

T_FRAMES = 36
LAM = 0.1
MAX_ITER = 100
N_SAMPLES = 64
DD = 384
P = 161
N_CORES = 8
SPC = N_SAMPLES // N_CORES          # samples per core
COLS = SPC * DD                     # 3072 columns per core
CHUNK = 512
NCH = COLS // CHUNK                 # 6 chunks
P1 = 128                            # K/M tile 1 rows
P2A = P - P1                        # 33  (y rows 128:161)
P2B = T_FRAMES                      # 36  (Ys rows)
P2 = P2A + P2B                      # 69  (K-tile 2 rows)
CONSTW = 3 * P + 2 * T_FRAMES       # packed constants width (555)

_CACHE = {}


def _host_prep(rr, theta):
    """dictionary, A-blocks, thresholds, momentum schedule (fp64 -> fp32)."""
    rr64 = rr.astype(np.float64)
    th64 = theta.astype(np.float64)
    i = np.arange(T_FRAMES, dtype=np.float64)[:, None]
    pw = rr64[None, :] ** i
    ang = i * th64[None, :]
    dic = np.concatenate(
        [np.ones((T_FRAMES, 1)), pw * np.cos(ang), pw * np.sin(ang)], axis=1
    ).astype(np.float32)                                     # [36, 161]

    DtD = (dic.astype(np.float64).T @ dic.astype(np.float64))
    Linv = 1.0 / np.linalg.norm(DtD, ord=2)
    A = (np.eye(P) - Linv * DtD).astype(np.float32)          # [161, 161]
    LD = (Linv * dic.astype(np.float64)).astype(np.float32)  # [36, 161]
    thr = np.float32(LAM * Linv)

    # momentum coefficients tt_k for k = 1..MAX_ITER+1 (tt_1 = 0)
    t = 1.0
    tt = [0.0]  # index 0 unused
    for _ in range(MAX_ITER + 1):
        t_new = (1.0 + np.sqrt(1.0 + 4.0 * t * t)) / 2.0
        tt.append((t - 1.0) / t_new)
        t = t_new

    lhs1 = np.ascontiguousarray(A[0:P1, :])                  # [128, 161]
    lhs2 = np.concatenate([A[P1:P, :], LD], axis=0)          # [69, 161]
    dicT = np.ascontiguousarray(dic.T)                       # [161, 36]
    rt1 = np.ascontiguousarray(dicT[0:P1, :])                # [128, 36]
    rt2 = np.ascontiguousarray(dicT[P1:P, :])                # [33, 36]

    # Pack every constant into one [128, CONSTW] block. Compute-engine reads
    # need 32-aligned partition bases, so the A-bottom block (33 rows) and the
    # Linv*D block (36 rows) each live at partition 0 of their own column
    # range; the DMA into the K-tile-2 weight/state tiles does the row shift.
    consts = np.zeros((P1, CONSTW), dtype=np.float32)
    consts[:, 0:P] = lhs1                                    # A rows 0:128
    consts[0:P2A, P:2 * P] = lhs2[0:P2A]                     # A rows 128:161
    consts[0:P2B, 2 * P:3 * P] = lhs2[P2A:P2]                # Linv * D
    consts[:, 3 * P:3 * P + T_FRAMES] = rt1
    consts[0:P2A, 3 * P + T_FRAMES:3 * P + 2 * T_FRAMES] = rt2
    return dic, consts, float(thr), tt


def _build_program():
    from contextlib import ExitStack

    import concourse.bass as bass
    import concourse.tile as tile
    from concourse import bacc, mybir

    f32 = mybir.dt.float32
    Alu = mybir.AluOpType
    Act = mybir.ActivationFunctionType

    nc = bacc.Bacc("TRN2", target_bir_lowering=False, debug=False)

    yin_d = nc.declare_dram_parameter("yin", [P2B, COLS], f32, isOutput=False)
    con_d = nc.declare_dram_parameter("consts", [P1, CONSTW], f32, isOutput=False)
    c_out = nc.declare_dram_parameter("c_out", [P, COLS], f32, isOutput=True)
    r_out = nc.declare_dram_parameter("r_out", [T_FRAMES, COLS], f32, isOutput=True)

    tt = _CACHE["tt"]
    thr = _CACHE["thr"]

    with tile.TileContext(nc) as tc, ExitStack() as ctx:
        const = ctx.enter_context(tc.tile_pool(name="const", bufs=1))
        state = ctx.enter_context(tc.tile_pool(name="state", bufs=1))
        xpool = ctx.enter_context(tc.tile_pool(name="xpool", bufs=2))
        spool = ctx.enter_context(tc.tile_pool(name="spool", bufs=2))
        cpool = ctx.enter_context(tc.tile_pool(name="cpool", bufs=4))
        wpool = ctx.enter_context(tc.tile_pool(name="wpool", bufs=2))
        rpool = ctx.enter_context(tc.tile_pool(name="rpool", bufs=3))
        psum = ctx.enter_context(tc.tile_pool(name="psum", bufs=3, space="PSUM"))

        conb = const.tile([P1, CONSTW], f32)
        lhs1 = conb[:, 0:P]
        lhs2a = conb[0:P2A, P:2 * P]          # A rows 128:161 at partition 0
        rt1 = conb[:, 3 * P:3 * P + T_FRAMES]
        rt2 = conb[0:P2A, 3 * P + T_FRAMES:3 * P + 2 * T_FRAMES]

        ytop = state.tile([P1, COLS], f32)
        ybot = state.tile([P2, COLS], f32)   # [0:33] = y rows 128:161, [33:69] = Ys
        # K-tile-2 weight slots (manual ping-pong): rows 0:33 get the scaled
        # A-bottom each iteration (ACT, partition base 0); rows 33:69 hold
        # Linv*D, DMA'd once (DMA shifts partitions, compute engines can't).
        sl2s = [
            state.tile([P2, P], f32, name=f"sl2_{i}", tag=f"sl2_{i}")
            for i in range(2)
        ]

        nc.sync.dma_start(conb[:], con_d[:])
        nc.sync.dma_start(ybot[P2A:P2, :], yin_d[:])
        for s2 in sl2s:
            nc.sync.dma_start(s2[P2A:P2, :], con_d[0:P2B, 2 * P:3 * P])
        nc.gpsimd.memset(ytop[:], 0.0)
        nc.gpsimd.memset(ybot[0:P2A, :], 0.0)
        tc.strict_bb_all_engine_barrier()

        xt_top = xt_bot = None
        xs_top_prev = xs_bot_prev = None

        for k in range(1, MAX_ITER + 1):
            # z_k = (1+tt_{k-1}) * A @ y~_{k-1} + B with y~ = x - s; the
            # (1+tt) scalar is folded into per-iteration scaled weight copies
            # (GPSIMD's Pool engine has no scalar_tensor_tensor opcode).
            om = 1.0 + tt[k - 1] if k >= 2 else 1.0
            sc = tt[k + 1] / (1.0 + tt[k + 1]) if k < MAX_ITER else 0.0
            sl1 = wpool.tile([P1, P], f32, tag="sl1")
            sl2 = sl2s[k % 2]
            nc.scalar.activation(sl1[:], lhs1, Act.Copy, bias=0.0, scale=om)
            nc.scalar.activation(sl2[0:P2A, :], lhs2a, Act.Copy, bias=0.0, scale=om)
            xt_top = xpool.tile([P1, COLS], f32, tag="xt_top")
            xt_bot = xpool.tile([P2A, COLS], f32, tag="xt_bot")
            if k < MAX_ITER:
                xs_top = spool.tile([P1, COLS], f32, tag="xs_top")
                xs_bot = spool.tile([P2A, COLS], f32, tag="xs_bot")
            for c in range(NCH):
                cs = bass.ts(c, CHUNK)
                pt = psum.tile([P1, CHUNK], f32, tag="pt")
                pb = psum.tile([P2A, CHUNK], f32, tag="pb")
                nc.tensor.matmul(pt[:], sl1[:, 0:P1], ytop[:, cs], start=True, stop=False)
                nc.tensor.matmul(pt[:], sl2[:, 0:P1], ybot[:, cs], start=False, stop=True)
                nc.tensor.matmul(pb[:], sl1[:, P1:P], ytop[:, cs], start=True, stop=False)
                nc.tensor.matmul(pb[:], sl2[:, P1:P], ybot[:, cs], start=False, stop=True)

                for (pz, xt, xs_cur, xs_prev, ydst, ctag, cpar) in (
                    (pt, xt_top, None if k == MAX_ITER else xs_top, xs_top_prev,
                     ytop[:, cs], "ct", P1),
                    (pb, xt_bot, None if k == MAX_ITER else xs_bot, xs_bot_prev,
                     ybot[0:P2A, cs], "cb", P2A),
                ):
                    cc = cpool.tile([cpar, CHUNK], f32, tag=ctag)
                    nc.vector.tensor_scalar(cc[:], pz[:], thr, -thr, Alu.min, Alu.max)
                    nc.vector.scalar_tensor_tensor(
                        xt[:, cs], pz[:], 1.0, cc[:], Alu.mult, Alu.subtract
                    )
                    if xs_cur is not None:
                        nc.scalar.activation(xs_cur[:, cs], xt[:, cs], Act.Copy,
                                             bias=0.0, scale=sc)
                    if k == 1:
                        nc.gpsimd.tensor_copy(ydst, xt[:, cs])
                    elif k < MAX_ITER:
                        nc.gpsimd.tensor_sub(ydst, xt[:, cs], xs_prev[:, cs])
            if k < MAX_ITER:
                xs_top_prev, xs_bot_prev = xs_top, xs_bot

        nc.sync.dma_start(c_out[0:P1, :], xt_top[:])
        nc.sync.dma_start(c_out[P1:P, :], xt_bot[:])

        # reconst = dic @ C  : [36, COLS]
        for c in range(NCH):
            cs = bass.ts(c, CHUNK)
            pr = psum.tile([T_FRAMES, CHUNK], f32, tag="pt")
            nc.tensor.matmul(pr[:], rt1[:], xt_top[:, cs], start=True, stop=False)
            nc.tensor.matmul(pr[:], rt2[:], xt_bot[:, cs], start=False, stop=True)
            rec = rpool.tile([T_FRAMES, CHUNK], f32, tag="rec")
            nc.vector.tensor_copy(rec[:], pr[:])
            nc.sync.dma_start(r_out[:, cs], rec[:])

    nc.compile()
    return nc


def _get_program(rr, theta):
    key = (rr.tobytes(), theta.tobytes())
    if _CACHE.get("key") != key:
        dic, consts, thr, tt = _host_prep(rr, theta)
        _CACHE.update(key=key, dic=dic, consts=consts, thr=thr, tt=tt, nc=None)
        _CACHE["nc"] = _build_program()
    return _CACHE["nc"]


def kernel(x, rr, theta):
    from concourse.bass_utils import run_bass_kernel_spmd

    x = np.asarray(x, dtype=np.float32)
    rr = np.asarray(rr, dtype=np.float32)
    theta = np.asarray(theta, dtype=np.float32)

    nc = _get_program(rr, theta)
    dic = _CACHE["dic"]

    in_maps = []
    for ci in range(N_CORES):
        ys = np.ascontiguousarray(
            x[ci * SPC:(ci + 1) * SPC].transpose(1, 0, 2).reshape(T_FRAMES, COLS)
        )
        in_maps.append({"yin": ys, "consts": _CACHE["consts"]})

    res = run_bass_kernel_spmd(nc, in_maps, list(range(N_CORES)))

    C = np.empty((N_SAMPLES, P, DD), dtype=np.float32)
    rec = np.empty((N_SAMPLES, T_FRAMES, DD), dtype=np.float32)
    for ci in range(N_CORES):
        co = res.results[ci]["c_out"].reshape(P, SPC, DD).transpose(1, 0, 2)
        ro = res.results[ci]["r_out"].reshape(T_FRAMES, SPC, DD).transpose(1, 0, 2)
        C[ci * SPC:(ci + 1) * SPC] = co
        rec[ci * SPC:(ci + 1) * SPC] = ro
    return C, dic, rec


# revision 22
# speedup vs baseline: 1.1377x; 1.1377x over previous
"""DyanEncoder kernel, hybrid fp32/f32r version.

Chunks 0-2 (1536 cols): plain fp32 matmuls (4 cyc/row, 4 MMs/chunk).
Chunks 3-5 (1536 cols): f32r 3-term hi/lo split (1 cyc/row, 12 MMs/chunk).
PE: 3*8192 + 3*6144 = 43008 cyc = 17.9 us/iter (vs 20.5 pure fp32).

Momentum via ybar = cs*x_prev - x (one DVE STT) with -(1+tt) folded into
per-iteration weight copies -- both the fp32 weight set and the f32r hi/lo
re-split set. This frees ACT (no s-pass) and GPSIMD (no y tensor_sub), whose
slack absorbs the f32r split passes and the ACT-relu softshrink:
  chunks 0-4: r1 = ACT relu(z-thr), r2 = ACT relu(-z-thr), x = r1-r2 (GPSIMD)
  chunk 5:    c = DVE clip, x = DVE STT   (keeps DVE/ACT balanced)
Measured-cost balance: DVE 16.6, ACT 18.2, GPSIMD 15.8, PE 17.9 us/iter.
"""

import numpy as np

T_FRAMES = 36
LAM = 0.1
MAX_ITER = 100
N_SAMPLES = 64
DD = 384
P = 161
N_CORES = 8
SPC = N_SAMPLES // N_CORES
COLS = SPC * DD                     # 3072
CHUNK = 512
NCH = COLS // CHUNK                 # 6
NF = 3                              # fp32 chunks 0..2
HCOLS = NF * CHUNK                  # 1536 per half
P1 = 128
P2A = P - P1                        # 33
P2B = T_FRAMES                      # 36
P2 = P2A + P2B                      # 69 (fp32 K2 rows)
KB = 64 + P2B                       # 100 (f32r K2 rows incl. aligned gap)
CONSTW = 5 * P                      # A_top | A_bot | LD_h | LD_l | LD_f32

_CACHE = {}


def _round11(a):
    a = np.asarray(a, np.float32)
    u = a.view(np.uint32).astype(np.uint64)
    low = u & np.uint64(0xFFF)
    half = np.uint64(0x800)
    u_hi = u & np.uint64(0xFFFFF000)
    rnd = np.where(
        (low > half)
        | ((low == half) & ((u >> np.uint64(12)) & np.uint64(1)).astype(bool)),
        u_hi + np.uint64(0x1000), u_hi,
    )
    return rnd.astype(np.uint32).view(np.float32)


def _host_prep(rr, theta):
    rr64 = rr.astype(np.float64)
    th64 = theta.astype(np.float64)
    i = np.arange(T_FRAMES, dtype=np.float64)[:, None]
    pw = rr64[None, :] ** i
    ang = i * th64[None, :]
    dic = np.concatenate(
        [np.ones((T_FRAMES, 1)), pw * np.cos(ang), pw * np.sin(ang)], axis=1
    ).astype(np.float32)

    DtD = dic.astype(np.float64).T @ dic.astype(np.float64)
    Linv = 1.0 / np.linalg.norm(DtD, ord=2)
    A = (np.eye(P) - Linv * DtD).astype(np.float32)
    LD = (Linv * dic.astype(np.float64)).astype(np.float32)
    LD_h = _round11(LD)
    LD_l = _round11(LD - LD_h)
    thr = np.float32(LAM * Linv)

    t = 1.0
    tt = [0.0]
    for _ in range(MAX_ITER + 1):
        t_new = (1.0 + np.sqrt(1.0 + 4.0 * t * t)) / 2.0
        tt.append((t - 1.0) / t_new)
        t = t_new

    consts = np.zeros((P1, CONSTW), dtype=np.float32)
    consts[:, 0:P] = A[0:P1, :]
    consts[0:P2A, P:2 * P] = A[P1:P, :]
    consts[64:KB, 2 * P:3 * P] = LD_h
    consts[64:KB, 3 * P:4 * P] = LD_l
    consts[0:P2B, 4 * P:5 * P] = LD

    dicT = np.ascontiguousarray(dic.T)
    rcon = np.zeros((P1, 2 * T_FRAMES), dtype=np.float32)
    rcon[:, 0:T_FRAMES] = dicT[0:P1, :]
    rcon[0:P2A, T_FRAMES:2 * T_FRAMES] = dicT[P1:P, :]
    return dic, consts, rcon, float(thr), tt


def _build_program():
    from contextlib import ExitStack

    import concourse.bass as bass
    import concourse.tile as tile
    from concourse import bacc, mybir

    f32 = mybir.dt.float32
    f32r = mybir.dt.float32r
    Alu = mybir.AluOpType
    Act = mybir.ActivationFunctionType

    nc = bacc.Bacc("TRN2", target_bir_lowering=False, debug=False)

    yin_d = nc.declare_dram_parameter("yin", [P2B, HCOLS], f32, isOutput=False)
    yinh_d = nc.declare_dram_parameter("yinh", [P2B, HCOLS], f32, isOutput=False)
    yinl_d = nc.declare_dram_parameter("yinl", [P2B, HCOLS], f32, isOutput=False)
    con_d = nc.declare_dram_parameter("consts", [P1, CONSTW], f32, isOutput=False)
    rcon_d = nc.declare_dram_parameter("rcon", [P1, 2 * T_FRAMES], f32, isOutput=False)
    c_out = nc.declare_dram_parameter("c_out", [P, COLS], f32, isOutput=True)
    r_out = nc.declare_dram_parameter("r_out", [T_FRAMES, COLS], f32, isOutput=True)

    tt = _CACHE["tt"]
    thr = _CACHE["thr"]

    with tile.TileContext(nc) as tc, ExitStack() as ctx:
        const = ctx.enter_context(tc.tile_pool(name="const", bufs=1))
        state = ctx.enter_context(tc.tile_pool(name="state", bufs=1))
        xpool = ctx.enter_context(tc.tile_pool(name="xpool", bufs=2))
        bpool = ctx.enter_context(tc.tile_pool(name="bpool", bufs=2))
        wpool = ctx.enter_context(tc.tile_pool(name="wpool", bufs=2))
        cpool = ctx.enter_context(tc.tile_pool(name="cpool", bufs=3))
        rpool = ctx.enter_context(tc.tile_pool(name="rpool", bufs=3))
        psum = ctx.enter_context(tc.tile_pool(name="psum", bufs=3, space="PSUM"))

        conb = const.tile([P1, CONSTW], f32)
        a_top = conb[:, 0:P]
        a_bot = conb[0:P2A, P:2 * P]
        ldh_s = conb[64:KB, 2 * P:3 * P]
        ldl_s = conb[64:KB, 3 * P:4 * P]
        rconb = const.tile([P1, 2 * T_FRAMES], f32)
        rt1 = rconb[:, 0:T_FRAMES]
        rt2 = rconb[0:P2A, T_FRAMES:2 * T_FRAMES]
        thrb = const.tile([P1, 1], f32)
        nc.gpsimd.memset(thrb[:], -thr)

        # fp32 half (chunks 0..2): moving state + Ys in rows 33:69
        ytf = state.tile([P1, HCOLS], f32)
        ybf = state.tile([P2, HCOLS], f32)
        # fp32 K2 weights ping-pong: rows 0:33 scaled A_bot, rows 33:69 LD
        wf2s = [state.tile([P2, P], f32, name=f"wf2_{i}", tag=f"wf2_{i}")
                for i in range(2)]

        # f32r half (chunks 3..5)
        yh_top = state.tile([P1, HCOLS], f32r)
        yl_top = state.tile([P1, HCOLS], f32r)
        kba = state.tile([KB, HCOLS], f32r)
        klb = state.tile([KB, HCOLS], f32r)
        wk4s = [state.tile([KB, P], f32r, name=f"wk4_{i}", tag=f"wk4_{i}")
                for i in range(2)]
        wk5s = [state.tile([KB, P], f32r, name=f"wk5_{i}", tag=f"wk5_{i}")
                for i in range(2)]
        ystage = const.tile([P1, 2 * HCOLS], f32)

        nc.sync.dma_start(conb[:], con_d[:])
        nc.sync.dma_start(rconb[:], rcon_d[:])
        nc.sync.dma_start(ybf[P2A:P2, :], yin_d[:])
        for wf2 in wf2s:
            nc.sync.dma_start(wf2[P2A:P2, :], con_d[0:P2B, 4 * P:5 * P])
        nc.sync.dma_start(ystage[64:KB, 0:HCOLS], yinh_d[:])
        nc.sync.dma_start(ystage[64:KB, HCOLS:2 * HCOLS], yinl_d[:])
        nc.vector.tensor_copy(kba[64:KB, :], ystage[64:KB, 0:HCOLS])
        nc.vector.tensor_copy(klb[64:KB, :], ystage[64:KB, HCOLS:2 * HCOLS])
        nc.gpsimd.memset(ytf[:], 0.0)
        nc.gpsimd.memset(ybf[0:P2A, :], 0.0)
        nc.vector.memset(ystage[:, 0:HCOLS], 0.0)
        nc.vector.tensor_copy(yh_top[:], ystage[:, 0:HCOLS])
        nc.gpsimd.tensor_copy(yl_top[:], ystage[:, 0:HCOLS])
        nc.vector.tensor_copy(kba[0:64, :], ystage[0:64, 0:HCOLS])
        nc.gpsimd.tensor_copy(klb[0:64, :], ystage[0:64, 0:HCOLS])
        for w4, w5 in zip(wk4s, wk5s):
            nc.vector.tensor_copy(w4[32:64, :], ystage[32:64, 0:P])
            nc.vector.tensor_copy(w5[32:64, :], ystage[32:64, 0:P])
            nc.vector.tensor_copy(w4[64:KB, :], ldh_s)
            nc.vector.tensor_copy(w5[64:KB, :], ldl_s)
        tc.strict_bb_all_engine_barrier()

        xt_top = xt_bot = None
        xp_top = xp_bot = None

        for k in range(1, MAX_ITER + 1):
            om = 1.0 + tt[k - 1]
            cs = tt[k] / (1.0 + tt[k])
            # fp32 weight set, scaled by -om
            wf1 = wpool.tile([P1, P], f32, tag="wf1")
            wf2 = wf2s[k % 2]
            nc.scalar.activation(wf1[:], a_top, Act.Copy, bias=0.0, scale=-om)
            nc.scalar.activation(wf2[0:P2A, :], a_bot, Act.Copy, bias=0.0, scale=-om)
            # f32r weight set: fresh hi/lo split of -om*A
            wh = wpool.tile([P1, P], f32r, tag="wh")
            wl = wpool.tile([P1, P], f32r, tag="wl")
            wk4 = wk4s[k % 2]
            wk5 = wk5s[k % 2]
            nc.scalar.activation(wh[:], a_top, Act.Copy, bias=0.0, scale=-om)
            nc.scalar.activation(wk4[0:P2A, :], a_bot, Act.Copy, bias=0.0, scale=-om)
            nc.vector.scalar_tensor_tensor(
                wl[:], a_top, -om, wh[:].bitcast(f32), Alu.mult, Alu.subtract)
            nc.vector.scalar_tensor_tensor(
                wk5[0:P2A, :], a_bot, -om, wk4[0:P2A, :].bitcast(f32),
                Alu.mult, Alu.subtract)

            xt_top = xpool.tile([P1, COLS], f32, tag="xt_top")
            xt_bot = xpool.tile([P2A, COLS], f32, tag="xt_bot")

            for c in range(NCH):
                g = bass.ts(c, CHUNK)                    # global col slice
                lc = bass.ts(c - NF, CHUNK) if c >= NF else bass.ts(c, CHUNK)
                pt = psum.tile([P1, CHUNK], f32, tag="pt")
                pb = psum.tile([P2A, CHUNK], f32, tag="pb")
                if c < NF:
                    for (pz, wcol) in ((pt, slice(0, P1)), (pb, slice(P1, P))):
                        nc.tensor.matmul(pz[:], wf1[:, wcol], ytf[:, lc], start=True, stop=False)
                        nc.tensor.matmul(pz[:], wf2[:, wcol], ybf[:, lc], start=False, stop=True)
                else:
                    for (pz, wcol) in ((pt, slice(0, P1)), (pb, slice(P1, P))):
                        nc.tensor.matmul(pz[:], wh[:, wcol], yh_top[:, lc], start=True, stop=False)
                        nc.tensor.matmul(pz[:], wh[:, wcol], yl_top[:, lc], start=False, stop=False)
                        nc.tensor.matmul(pz[:], wl[:, wcol], yh_top[:, lc], start=False, stop=False)
                        nc.tensor.matmul(pz[:], wk4[:, wcol], kba[:, lc], start=False, stop=False)
                        nc.tensor.matmul(pz[:], wk5[:, wcol], kba[:, lc], start=False, stop=False)
                        nc.tensor.matmul(pz[:], wk4[:, wcol], klb[:, lc], start=False, stop=True)

                for (pz, xt, xp, npar, ytag) in (
                    (pt, xt_top, xp_top, P1, "t"),
                    (pb, xt_bot, xp_bot, P2A, "b"),
                ):
                    # softshrink: ACT-relu pair on chunks 0-4, DVE clip on 5
                    if c < NCH - 1:
                        r1 = cpool.tile([npar, CHUNK], f32, tag="r1" + ytag)
                        r2 = cpool.tile([npar, CHUNK], f32, tag="r2" + ytag)
                        nc.scalar.activation(r1[:], pz[:], Act.Relu,
                                             bias=thrb[0:npar, :], scale=1.0)
                        nc.scalar.activation(r2[:], pz[:], Act.Relu,
                                             bias=thrb[0:npar, :], scale=-1.0)
                        nc.gpsimd.tensor_sub(xt[:, g], r1[:], r2[:])
                    else:
                        cc = cpool.tile([npar, CHUNK], f32, tag="cc" + ytag)
                        nc.vector.tensor_scalar(cc[:], pz[:], thr, -thr,
                                                Alu.min, Alu.max)
                        nc.vector.scalar_tensor_tensor(
                            xt[:, g], pz[:], 1.0, cc[:], Alu.mult, Alu.subtract)

                    if k == MAX_ITER:
                        continue
                    # ybar destination: fp32 state directly, or split via tmp
                    if c < NF:
                        ydst = ytf[:, lc] if npar == P1 else ybf[0:P2A, lc]
                        if k == 1:
                            nc.vector.tensor_scalar_mul(ydst, xt[:, g], -1.0)
                        else:
                            nc.vector.scalar_tensor_tensor(
                                ydst, xp[:, g], cs, xt[:, g], Alu.mult, Alu.subtract)
                    else:
                        yb = bpool.tile([npar, CHUNK], f32, tag="yb" + ytag)
                        if k == 1:
                            nc.vector.tensor_scalar_mul(yb[:], xt[:, g], -1.0)
                        else:
                            nc.vector.scalar_tensor_tensor(
                                yb[:], xp[:, g], cs, xt[:, g], Alu.mult, Alu.subtract)
                        yhdst = yh_top[:, lc] if npar == P1 else kba[0:P2A, lc]
                        yldst = yl_top[:, lc] if npar == P1 else klb[0:P2A, lc]
                        nc.vector.tensor_copy(yhdst, yb[:])
                        nc.vector.scalar_tensor_tensor(
                            yldst, yb[:], 1.0, yhdst.bitcast(f32),
                            Alu.mult, Alu.subtract)
            xp_top, xp_bot = xt_top, xt_bot

        nc.sync.dma_start(c_out[0:P1, :], xt_top[:])
        nc.sync.dma_start(c_out[P1:P, :], xt_bot[:])

        for c in range(NCH):
            g = bass.ts(c, CHUNK)
            pr = psum.tile([T_FRAMES, CHUNK], f32, tag="pt")
            nc.tensor.matmul(pr[:], rt1, xt_top[:, g], start=True, stop=False)
            nc.tensor.matmul(pr[:], rt2, xt_bot[:, g], start=False, stop=True)
            rec = rpool.tile([T_FRAMES, CHUNK], f32, tag="rec")
            nc.vector.tensor_copy(rec[:], pr[:])
            nc.sync.dma_start(r_out[:, g], rec[:])

    nc.compile()
    return nc


def _get_program(rr, theta):
    key = (rr.tobytes(), theta.tobytes())
    if _CACHE.get("key") != key:
        dic, consts, rcon, thr, tt = _host_prep(rr, theta)
        _CACHE.update(key=key, dic=dic, consts=consts, rcon=rcon,
                      thr=thr, tt=tt, nc=None)
        _CACHE["nc"] = _build_program()
    return _CACHE["nc"]


def kernel(x, rr, theta):
    from concourse.bass_utils import run_bass_kernel_spmd

    x = np.asarray(x, dtype=np.float32)
    rr = np.asarray(rr, dtype=np.float32)
    theta = np.asarray(theta, dtype=np.float32)

    nc = _get_program(rr, theta)
    dic = _CACHE["dic"]

    in_maps = []
    for ci in range(N_CORES):
        ys = np.ascontiguousarray(
            x[ci * SPC:(ci + 1) * SPC].transpose(1, 0, 2).reshape(T_FRAMES, COLS)
        )
        ysr = ys[:, HCOLS:]
        ysh = _round11(ysr)
        ysl = _round11(ysr - ysh)
        in_maps.append({
            "yin": np.ascontiguousarray(ys[:, 0:HCOLS]),
            "yinh": ysh, "yinl": ysl,
            "consts": _CACHE["consts"], "rcon": _CACHE["rcon"],
        })

    res = run_bass_kernel_spmd(nc, in_maps, list(range(N_CORES)))

    C = np.empty((N_SAMPLES, P, DD), dtype=np.float32)
    rec = np.empty((N_SAMPLES, T_FRAMES, DD), dtype=np.float32)
    for ci in range(N_CORES):
        co = res.results[ci]["c_out"].reshape(P, SPC, DD).transpose(1, 0, 2)
        ro = res.results[ci]["r_out"].reshape(T_FRAMES, SPC, DD).transpose(1, 0, 2)
        C[ci * SPC:(ci + 1) * SPC] = co
        rec[ci * SPC:(ci + 1) * SPC] = ro
    return C, dic, rec
